# revision 20
# baseline (speedup 1.0000x reference)
"""Causal self-attention on 8 Trainium2 NeuronCores.

Problem: B=2, T=2048, C=1024, 16 heads x 64 dim, fp32.

Sharding: tensor-parallel over heads x data-parallel over batch.
Each core owns one batch element (cores 0-3 -> b=0, 4-7 -> b=1) and a
group of 4 consecutive heads. Each core computes:
  - QKV projection for its 4 heads (producing qT/kT transposed, V natural)
  - causal attention for its 4 heads (scores kept transposed: ST[tk, tq])
  - partial output projection (its heads' rows of w_proj)
The host sums the 4 partial projections per batch and adds b_proj.

All device I/O is bf16 (half the DMA bytes of fp32); accumulation stays
in fp32 PSUM. The schedule is a single fused pipeline: attention is
ACT-bound (exp), so QKV-projection blocks, V-projection tiles and
output-projection tiles are interleaved as PE filler quanta *throughout*
the attention steady loops, keeping the in-order PE queue busy while ACT
computes exp. Attention processes diagonal tk tiles first (their mask
multiplies, on the otherwise-idle GpSimd, get slack), ST is issued 2
tiles ahead and exp 1 ahead of PV. Dummy matmuls during the initial DMA
wait ramp the PE p-state; the first two QKV blocks run split-N so they
can start on half-chunk transfers.

Softmax denominators: the ones-column of [V|1] gives row DH of each PV
accumulator; the chain is copy-to-SBUF -> reciprocal -> PE matmul with
lhsT=ones broadcasts the reciprocal row into the accumulator's unused
partitions 64..127 -> one DVE multiply normalizes into yT.

Device layouts (per core, bf16 in DRAM unless noted):
  xT    [1024, 2048]  x[b] transposed (channels on partitions)
  wqk   [1024, 512]   cols: q(h0)|q(h1)|k(h0)|k(h1)|q(h2)|q(h3)|k(h2)|k(h3)
  wv    [1024, 256]   v cols of the 4 heads
  wo    [256, 1024]   w_proj rows of the 4 heads
  bqkT  [128, 4] f32  cols: pair0-q, pair0-k, pair1-q, pair1-k biases
  bvr   [128, 256]    v bias of the 4 heads, replicated over partitions
  mask2 [128, 256]    two copies of mask[i,j] = 1 if i<=j else 0
  out   [2048, 1024]  partial (pre-bias) output projection, bf16
"""

import numpy as np

B, T, C = 2, 2048, 1024
NH, DH = 16, 64
NCORES = 8
HPC = 4  # heads per core
P = 128
CK = C // P  # 8 contraction tiles over channels
NT = T // P  # 16 token tiles
SLAB = 512
NSL = T // SLAB  # 4 tq slabs

_CACHE = {}


def _build_program():
    from contextlib import ExitStack

    import concourse.bacc as bacc
    import concourse.tile as tile
    from concourse import mybir

    f32 = mybir.dt.float32
    f32r = mybir.dt.float32r
    bf16 = mybir.dt.bfloat16
    AF = mybir.ActivationFunctionType

    nc = bacc.Bacc(
        "TRN2", target_bir_lowering=False, debug=False, num_devices=NCORES
    )

    xT = nc.dram_tensor("xT", [C, T], bf16, kind="ExternalInput").ap()
    wqk = nc.dram_tensor("wqk", [C, 4 * P], bf16, kind="ExternalInput").ap()
    wv = nc.dram_tensor("wv", [C, HPC * DH], bf16, kind="ExternalInput").ap()
    wo = nc.dram_tensor("wo", [HPC * DH, C], bf16, kind="ExternalInput").ap()
    bqkT = nc.dram_tensor("bqkT", [P, 4], f32, kind="ExternalInput").ap()
    bvr = nc.dram_tensor("bvr", [P, 2 * HPC * DH], bf16, kind="ExternalInput").ap()
    mask2 = nc.dram_tensor("mask2", [P, 2 * P], bf16, kind="ExternalInput").ap()
    out = nc.dram_tensor("out", [T, C], bf16, kind="ExternalOutput").ap()

    with tile.TileContext(nc) as tc, ExitStack() as ctx:
        const = ctx.enter_context(tc.tile_pool(name="const", bufs=1))
        # PSUM budget: 3 x [128,1024] (6 banks) shared by QKV/ST/outproj
        # + 2 x [128,512] (2 banks) for the PV accumulators = 8 banks.
        stp = ctx.enter_context(tc.tile_pool(name="stp", bufs=3, space="PSUM"))
        yp = ctx.enter_context(tc.tile_pool(name="yp", bufs=2, space="PSUM"))
        expp = ctx.enter_context(tc.tile_pool(name="expp", bufs=4))
        rbp = ctx.enter_context(tc.tile_pool(name="rbp", bufs=2))
        outp = ctx.enter_context(tc.tile_pool(name="outp", bufs=3))

        x_ch = [
            const.tile([P, CK, SLAB], bf16, name=f"x_ch{c}") for c in range(NSL)
        ]
        wqk_sb = const.tile([P, CK, 4 * P], bf16, name="wqk_sb")
        wv_sb = const.tile([P, CK, HPC * DH], bf16, name="wv_sb")
        wo_sb = const.tile([P, 2, C], bf16, name="wo_sb")
        bqk_sb = const.tile([P, 4], f32, name="bqk_sb")
        bv_sb = const.tile([P, 2, HPC, DH], bf16, name="bv_sb")
        mask2_sb = const.tile([P, 2, P], bf16, name="mask2_sb")
        v_sb = const.tile([P, NT, HPC, DH + 1], bf16, name="v_sb")
        warm = const.tile([P, 2], f32, name="warm")
        ones64 = const.tile([1, DH], f32, name="ones64")
        dumm = const.tile([P, SLAB], bf16, name="dumm")
        qT = [const.tile([P, T], bf16, name=f"qT{p}") for p in range(2)]
        kT = [const.tile([P, T], bf16, name=f"kT{p}") for p in range(2)]
        yT = [const.tile([P, T], bf16, name=f"yT{p}") for p in range(2)]

        # --- loads: wqk/x(slab0) per-chunk HALF transfers interleaved so
        # the split-N first blocks start earliest; the rest after; x slabs
        # 1-3 last. bqkT/bvr/mask2 are host-prepped (no transposing or
        # broadcasting DMA descriptors -- those are pathologically slow) ---
        wqkv = wqk.rearrange("(k p) n -> k p n", p=P)
        xTv = xT.rearrange("(k p) t -> k p t", p=P)
        H = SLAB // 2
        for k in range(CK):
            nc.sync.dma_start(out=wqk_sb[:, k, 0 : 2 * P], in_=wqkv[k][:, 0 : 2 * P])
            nc.sync.dma_start(out=x_ch[0][:, k, 0:H], in_=xTv[k][:, 0:H])
        wvv = wv.rearrange("(k p) n -> k p n", p=P)
        for k in range(CK):
            nc.sync.dma_start(out=x_ch[0][:, k, H:SLAB], in_=xTv[k][:, H:SLAB])
            nc.sync.dma_start(out=wqk_sb[:, k, 2 * P : 4 * P], in_=wqkv[k][:, 2 * P : 4 * P])
            nc.sync.dma_start(out=wv_sb[:, k, :], in_=wvv[k])
        nc.sync.dma_start(out=bqk_sb[:], in_=bqkT)
        nc.sync.dma_start(
            out=bv_sb[:], in_=bvr.rearrange("p (u h d) -> p u h d", d=DH, h=HPC)
        )
        nc.sync.dma_start(
            out=mask2_sb[:], in_=mask2.rearrange("p (h n) -> p h n", n=P)
        )
        nc.sync.dma_start(out=wo_sb[:], in_=wo.rearrange("(r p) n -> p r n", p=P))
        for c in range(1, NSL):
            for k in range(CK):
                nc.sync.dma_start(
                    out=x_ch[c][:, k, :],
                    in_=xTv[k][:, c * SLAB : (c + 1) * SLAB],
                )

        # scratch init + ACT exp-table preload + PE p-state warmup, all
        # during the initial DMA wait
        nc.vector.memset(warm[:, 0:1], 0.0)
        nc.scalar.activation(
            out=warm[:, 1:2], in_=warm[:, 0:1], func=AF.Exp,
            scale=float(1.0 / np.sqrt(DH)),
        )
        nc.vector.memset(ones64[:], 1.0)
        nc.vector.memset(dumm[:], 0.0)
        nc.vector.memset(v_sb[:, :, :, DH : DH + 1], 1.0)
        wps = stp.tile([P, 2 * SLAB], f32, name="wps", tag="big")
        for _ in range(12):
            nc.tensor.matmul(
                wps[:, :SLAB], lhsT=dumm[:, 0:P], rhs=dumm[:],
                start=True, stop=True,
            )
        nc.vector.tensor_copy(out=dumm[0:1, 0:1], in_=wps[0:1, 0:1])

        # --- filler quanta ----------------------------------------------
        # One PSUM tile per quantum (pair of blocks / pair of V tiles) so
        # stp-pool rotation couples to far fewer DVE consumers.
        def qk_pair(s, p, split=False, on_act=False):
            """QKV projection for the q AND k blocks of pair p, slab s,
            in the two halves of one PSUM tile. Bias apply + PSUM->SBUF
            copies on DVE (or ACT while it is still exp-idle).
            split=True runs N=256 groups so the first mms only need
            half-chunk DMAs."""
            ps = stp.tile([P, 2 * SLAB], f32, name="ps_qkv", tag="big")
            cols = [(0, H), (H, SLAB)] if split else [(0, SLAB)]
            for qk in range(2):
                blk = 2 * p + qk
                for c0, c1 in cols:
                    for k in range(CK):
                        nc.tensor.matmul(
                            ps[:, qk * SLAB + c0 : qk * SLAB + c1],
                            lhsT=wqk_sb[:, k, blk * P : (blk + 1) * P],
                            rhs=x_ch[s][:, k, c0:c1],
                            start=(k == 0),
                            stop=(k == CK - 1),
                        )
            for qk in range(2):
                blk = 2 * p + qk
                dst = qT[p] if qk == 0 else kT[p]
                sl = ps[:, qk * SLAB : (qk + 1) * SLAB]
                if on_act:
                    nc.scalar.activation(
                        out=dst[:, s * SLAB : (s + 1) * SLAB],
                        in_=sl,
                        func=AF.Identity,
                        bias=bqk_sb[:, blk : blk + 1],
                        scale=1.0,
                    )
                else:
                    nc.vector.tensor_scalar_add(
                        out=dst[:, s * SLAB : (s + 1) * SLAB],
                        in0=sl,
                        scalar1=bqk_sb[:, blk : blk + 1],
                    )

        def v_pair(s, a):
            """V projection for token tiles (2a, 2a+1) of slab s in one
            PSUM tile, one fused bias add (ones column at DH preset by
            memset)."""
            t0 = 4 * s + 2 * a
            ps = stp.tile([P, 2 * SLAB], f32, name="ps_v", tag="big")[
                :, : 2 * HPC * DH
            ]
            for u in range(2):
                for k in range(CK):
                    nc.tensor.matmul(
                        ps[:, u * HPC * DH : (u + 1) * HPC * DH],
                        lhsT=x_ch[s][:, k, (2 * a + u) * P : (2 * a + u + 1) * P],
                        rhs=wv_sb[:, k, :],
                        start=(k == 0),
                        stop=(k == CK - 1),
                    )
            nc.vector.tensor_add(
                out=v_sb[:, t0 : t0 + 2, :, 0:DH],
                in0=ps[:].rearrange("p (u h d) -> p u h d", d=DH, h=HPC),
                in1=bv_sb[:],
            )

        def proj_tile(s, tt):
            """Output projection for one token tile of slab s (partial
            over this core's heads). Pair-0 matmuls first: they can run
            while pair 1's softmax denominators are still resolving."""
            t = 4 * s + tt
            pso = stp.tile([P, 2 * SLAB], f32, name="pso", tag="big")
            for p in range(2):
                for ns in range(2):
                    nc.tensor.matmul(
                        pso[:, ns * SLAB : (ns + 1) * SLAB],
                        lhsT=yT[p][:, t * P : (t + 1) * P],
                        rhs=wo_sb[:, p, ns * SLAB : (ns + 1) * SLAB],
                        start=(p == 0),
                        stop=(p == 1),
                    )
            ob = outp.tile([P, 2 * SLAB], bf16, name="ob", tag="ob")
            nc.vector.tensor_copy(out=ob[:], in_=pso[:])
            nc.sync.dma_start(out=out[t * P : (t + 1) * P, :], in_=ob[:])

        def chain(p, s, psy):
            """Softmax denominator: copy both heads' denominator rows to
            SBUF, one reciprocal, PE-broadcast each into the unused
            partitions 64..127 of its PV accumulator, then normalize into
            yT with a single DVE multiply per head."""
            for hp in range(2):
                sm = rbp.tile([1, SLAB], f32, name="sm", tag="sm")
                nc.vector.tensor_copy(out=sm[:], in_=psy[hp][DH : DH + 1, :])
                rec = rbp.tile([1, SLAB], f32, name="rec", tag="rec")
                nc.vector.reciprocal_approx_fast(out=rec[:], in_=sm[:])
                rb = rbp.tile([DH, SLAB], f32, name="rb", tag="rb")
                nc.gpsimd.partition_broadcast(out_ap=rb[:], in_ap=rec[:])
                nc.vector.tensor_mul(
                    out=yT[p][hp * DH : (hp + 1) * DH, s * SLAB : (s + 1) * SLAB],
                    in0=psy[hp][0:DH, :],
                    in1=rb[:],
                )

        def attn(p, s, fillers, n_warm=2):
            """Causal attention for head pair p, tq slab s. Returns a
            closure that emits the softmax-denominator chain (so it can
            be deferred past other work).

            tk order puts the diagonal tiles first: the first executed tk
            (4s) covers the full [0:SLAB] psum width with start=True, the
            other diagonal tiles accumulate partial widths, then the full
            tiles 0..4s-1. The first `n_warm` fillers are emitted right
            after the ST/exp warmup; the rest are spread evenly through
            the steady loop to keep the PE fed while ACT runs exp.
            """
            ntk = 4 * s + 4
            tk_order = [4 * s + i for i in range(4)] + list(range(4 * s))
            psy = [
                yp.tile([P, SLAB], f32, name=f"psy{hp}", tag="psy")
                for hp in range(2)
            ]

            def off_of(tk):
                diag_r = tk - 4 * s
                return diag_r * P if diag_r >= 0 else 0

            pend = {}
            exd = {}

            def st(i):
                tk = tk_order[i]
                off = off_of(tk)
                pp = stp.tile([P, 2 * SLAB], f32, name="pp", tag="big")
                for hp in range(2):
                    nc.tensor.matmul(
                        pp[:, hp * SLAB + off : (hp + 1) * SLAB],
                        lhsT=kT[p][hp * DH : (hp + 1) * DH, tk * P : (tk + 1) * P],
                        rhs=qT[p][hp * DH : (hp + 1) * DH, s * SLAB + off : (s + 1) * SLAB],
                        start=True,
                        stop=True,
                    )
                pend[i] = pp

            def do_exp(i):
                tk = tk_order[i]
                off = off_of(tk)
                pp = pend.pop(i)
                ex = expp.tile([P, 2 * SLAB], bf16, name="ex", tag="ex")
                ppv = pp[:].rearrange("q (h n) -> q h n", h=2)[:, :, off:]
                exv = ex[:].rearrange("q (h n) -> q h n", h=2)[:, :, off:]
                nc.scalar.activation(
                    out=exv,
                    in_=ppv,
                    func=AF.Exp,
                    scale=float(1.0 / np.sqrt(DH)),
                )
                if tk - 4 * s >= 0:
                    exd2 = ex[:].rearrange("q (h n) -> q h n", h=2)[
                        :, :, off : off + P
                    ]
                    nc.vector.tensor_mul(out=exd2, in0=exd2, in1=mask2_sb[:])
                exd[i] = ex

            for i in range(min(3, ntk)):
                st(i)
            do_exp(0)
            if ntk > 1:
                do_exp(1)
            n_warm = min(n_warm, len(fillers))
            for f in fillers[:n_warm]:
                f()
            rest = fillers[n_warm:]
            pos = {}
            for j in range(len(rest)):
                pos.setdefault((j + 1) * ntk // (len(rest) + 1), []).append(
                    rest[j]
                )
            for i in range(ntk):
                tk = tk_order[i]
                off = off_of(tk)
                if i + 3 < ntk:
                    st(i + 3)
                if i + 2 < ntk:
                    do_exp(i + 2)
                ex = exd.pop(i)
                for hp in range(2):
                    nc.tensor.matmul(
                        psy[hp][0 : DH + 1, off:],
                        lhsT=v_sb[:, tk, 2 * p + hp, :],
                        rhs=ex[:, hp * SLAB + off : (hp + 1) * SLAB],
                        start=(i == 0),
                        stop=(i == ntk - 1),
                    )
                for f in pos.get(i, []):
                    f()
            return lambda: chain(p, s, psy)

        # --- fused schedule ---------------------------------------------
        # QKV blocks / V tiles / proj tiles of neighbouring slabs ride as
        # filler inside the ACT-bound attention loops. V tiles of slab s+1
        # sit between the two pair-1 chains' producers and consumers so
        # the deferred chain overlaps PE work.
        QP, VP, O = qk_pair, v_pair, proj_tile
        qk_pair(0, 0, split=True, on_act=True)
        v_pair(0, 0)
        c = attn(0, 0, [lambda: VP(0, 1), lambda: QP(0, 1)])
        c()
        c = attn(1, 0, [lambda: QP(1, 0)], n_warm=1)
        c()
        v_pair(1, 0)
        v_pair(1, 1)
        c = attn(0, 1, [lambda: O(0, 0), lambda: O(0, 1), lambda: O(0, 2),
                        lambda: O(0, 3), lambda: QP(1, 1)])
        c()
        c = attn(1, 1, [lambda: QP(2, 0)], n_warm=1)
        c()
        v_pair(2, 0)
        v_pair(2, 1)
        c = attn(0, 2, [lambda: O(1, 0), lambda: O(1, 1), lambda: O(1, 2),
                        lambda: O(1, 3), lambda: QP(2, 1)])
        c()
        c = attn(1, 2, [lambda: QP(3, 0)], n_warm=1)
        c()
        v_pair(3, 0)
        c = attn(0, 3, [lambda: VP(3, 1), lambda: O(2, 0),
                        lambda: O(2, 1), lambda: QP(3, 1)], n_warm=1)
        c()
        c = attn(1, 3, [lambda: O(2, 2), lambda: O(2, 3)], n_warm=1)
        c()
        proj_tile(3, 0)
        proj_tile(3, 1)
        proj_tile(3, 2)
        proj_tile(3, 3)

    nc.compile()
    return nc


def get_program():
    if "nc" not in _CACHE:
        _CACHE["nc"] = _build_program()
    return _CACHE["nc"]


def make_core_inputs(x, w_attn, b_attn, w_proj, core):
    """Host-side shard preparation for one core (bf16 device I/O)."""
    import ml_dtypes

    bf16 = ml_dtypes.bfloat16
    b = core // 4
    g = core % 4
    heads = [4 * g + i for i in range(HPC)]

    xT = np.ascontiguousarray(np.asarray(x[b], np.float32).T).astype(bf16)

    def qcols(h):
        return w_attn[:, h * DH : (h + 1) * DH]

    def kcols(h):
        return w_attn[:, C + h * DH : C + (h + 1) * DH]

    def vcols(h):
        return w_attn[:, 2 * C + h * DH : 2 * C + (h + 1) * DH]

    h0, h1, h2, h3 = heads
    wqk = np.ascontiguousarray(
        np.concatenate(
            [qcols(h0), qcols(h1), kcols(h0), kcols(h1),
             qcols(h2), qcols(h3), kcols(h2), kcols(h3)],
            axis=1,
        )
    ).astype(bf16)
    wv = np.ascontiguousarray(
        np.concatenate([vcols(h) for h in heads], axis=1)
    ).astype(bf16)
    bqkT = np.stack(
        [
            np.concatenate([b_attn[h0 * DH : (h0 + 1) * DH], b_attn[h1 * DH : (h1 + 1) * DH]]),
            np.concatenate([b_attn[C + h0 * DH : C + (h0 + 1) * DH], b_attn[C + h1 * DH : C + (h1 + 1) * DH]]),
            np.concatenate([b_attn[h2 * DH : (h2 + 1) * DH], b_attn[h3 * DH : (h3 + 1) * DH]]),
            np.concatenate([b_attn[C + h2 * DH : C + (h2 + 1) * DH], b_attn[C + h3 * DH : C + (h3 + 1) * DH]]),
        ],
        axis=1,
    ).astype(np.float32)
    bv = np.concatenate(
        [b_attn[2 * C + h * DH : 2 * C + (h + 1) * DH] for h in heads]
    ).astype(bf16)
    bvr = np.broadcast_to(
        np.concatenate([bv, bv])[None, :], (P, 2 * HPC * DH)
    )
    wo = np.ascontiguousarray(
        w_proj[heads[0] * DH : (heads[-1] + 1) * DH, :]
    ).astype(bf16)

    mask = np.triu(np.ones((P, P))).astype(bf16)
    mask2 = np.concatenate([mask, mask], axis=1)
    return {
        "xT": xT,
        "wqk": wqk,
        "wv": wv,
        "wo": wo,
        "bqkT": np.ascontiguousarray(bqkT),
        "bvr": np.ascontiguousarray(bvr),
        "mask2": np.ascontiguousarray(mask2),
    }


def kernel(x, w_attn, b_attn, w_proj, b_proj):
    from concourse.bass_utils import run_bass_kernel_spmd

    x = np.asarray(x, np.float32)
    w_attn = np.asarray(w_attn, np.float32)
    b_attn = np.asarray(b_attn, np.float32)
    w_proj = np.asarray(w_proj, np.float32)
    b_proj = np.asarray(b_proj, np.float32)

    nc = get_program()
    in_maps = [
        make_core_inputs(x, w_attn, b_attn, w_proj, core) for core in range(NCORES)
    ]
    res = run_bass_kernel_spmd(nc, in_maps, core_ids=list(range(NCORES)))
    outs = [np.asarray(m["out"], np.float32) for m in res.results]

    y = np.empty((B, T, C), np.float32)
    for b in range(B):
        y[b] = outs[4 * b] + outs[4 * b + 1] + outs[4 * b + 2] + outs[4 * b + 3]
        y[b] += b_proj[None, :]
    return y


# revision 21
# speedup vs baseline: 1.0549x; 1.0549x over previous
"""Causal self-attention on 8 Trainium2 NeuronCores.

Problem: B=2, T=2048, C=1024, 16 heads x 64 dim, fp32.

Sharding: tensor-parallel over heads x data-parallel over batch.
Each core owns one batch element (cores 0-3 -> b=0, 4-7 -> b=1) and a
group of 4 consecutive heads. Each core computes:
  - QKV projection for its 4 heads (producing qT/kT transposed, V natural)
  - causal attention for its 4 heads (scores kept transposed: ST[tk, tq])
  - partial output projection (its heads' rows of w_proj)
The host sums the 4 partial projections per batch and adds b_proj.

All device I/O is bf16 (half the DMA bytes of fp32); accumulation stays
in fp32 PSUM. The schedule is a single fused pipeline: attention is
ACT-bound (exp), so QKV-projection blocks, V-projection tiles and
output-projection tiles are interleaved as PE filler quanta *throughout*
the attention steady loops, keeping the in-order PE queue busy while ACT
computes exp. Attention processes diagonal tk tiles first (their fused
two-head mask multiply gets slack), ST is issued 3 tiles ahead and exp 2
ahead of PV.

Device layouts (per core, bf16 in DRAM unless noted):
  xT    [1024, 2048]  x[b] transposed (channels on partitions)
  wqk   [1024, 512]   cols: q(h0)|q(h1)|k(h0)|k(h1)|q(h2)|q(h3)|k(h2)|k(h3)
  wv    [1024, 256]   v cols of the 4 heads
  wo    [256, 1024]   w_proj rows of the 4 heads
  bqkT  [128, 4] f32  cols: pair0-q, pair0-k, pair1-q, pair1-k biases
  bvr   [128, 256]    v bias of the 4 heads, replicated over partitions
  mask2 [128, 256]    two copies of mask[i,j] = 1 if i<=j else 0
  out   [2048, 1024]  partial (pre-bias) output projection, bf16

Attention math per head (pair tiles hold 2 heads at partitions 0-63/64-127):
  qT/kT [64, T] from matmul(lhsT=w_cols, rhs=xT)       (K=C, N=T slabs)
  ST    [tk, tq] = matmul(lhsT=kT tile, rhs=qT slab)   (K=64)
  expST = exp(0.125 * ST) on ACT, diagonal blocks masked by multiply
  yT_ext[65, tq] = matmul(lhsT=[V|1] tile, rhs=expST)  accumulated over tk
  yT = yT_ext[0:64] * reciprocal(yT_ext[64])           (softmax denominator)
  out += matmul(lhsT=yT tiles, rhs=wo)                 (K=256)
Causality: tk tiles > tq slab are skipped entirely; diagonal tk tiles only
compute columns tq >= tile start (partial-N matmuls).
"""

import numpy as np

B, T, C = 2, 2048, 1024
NH, DH = 16, 64
NCORES = 8
HPC = 4  # heads per core
P = 128
CK = C // P  # 8 contraction tiles over channels
NT = T // P  # 16 token tiles
SLAB = 512
NSL = T // SLAB  # 4 tq slabs

_CACHE = {}


def _build_program():
    from contextlib import ExitStack

    import concourse.bacc as bacc
    import concourse.tile as tile
    from concourse import mybir

    f32 = mybir.dt.float32
    bf16 = mybir.dt.bfloat16
    AF = mybir.ActivationFunctionType

    nc = bacc.Bacc(
        "TRN2", target_bir_lowering=False, debug=False, num_devices=NCORES
    )

    xT = nc.dram_tensor("xT", [C, T], bf16, kind="ExternalInput").ap()
    wqk = nc.dram_tensor("wqk", [C, 4 * P], bf16, kind="ExternalInput").ap()
    wv = nc.dram_tensor("wv", [C, HPC * DH], bf16, kind="ExternalInput").ap()
    wo = nc.dram_tensor("wo", [HPC * DH, C], bf16, kind="ExternalInput").ap()
    bqkT = nc.dram_tensor("bqkT", [P, 4], f32, kind="ExternalInput").ap()
    bvr = nc.dram_tensor("bvr", [P, HPC * DH], bf16, kind="ExternalInput").ap()
    mask2 = nc.dram_tensor("mask2", [P, 2 * P], bf16, kind="ExternalInput").ap()
    out = nc.dram_tensor("out", [T, C], bf16, kind="ExternalOutput").ap()

    with tile.TileContext(nc) as tc, ExitStack() as ctx:
        const = ctx.enter_context(tc.tile_pool(name="const", bufs=1))
        # PSUM budget: 3 x [128,1024] (6 banks) shared by QKV/ST/outproj
        # + 2 x [128,512] (2 banks) for the PV accumulators = 8 banks.
        stp = ctx.enter_context(tc.tile_pool(name="stp", bufs=3, space="PSUM"))
        yp = ctx.enter_context(tc.tile_pool(name="yp", bufs=2, space="PSUM"))
        expp = ctx.enter_context(tc.tile_pool(name="expp", bufs=4))
        rbp = ctx.enter_context(tc.tile_pool(name="rbp", bufs=2))
        outp = ctx.enter_context(tc.tile_pool(name="outp", bufs=3))

        x_ch = [
            const.tile([P, CK, SLAB], bf16, name=f"x_ch{c}") for c in range(NSL)
        ]
        wqk_sb = const.tile([P, CK, 4 * P], bf16, name="wqk_sb")
        wv_sb = const.tile([P, CK, HPC * DH], bf16, name="wv_sb")
        wo_sb = const.tile([P, 2, C], bf16, name="wo_sb")
        bqk_sb = const.tile([P, 4], f32, name="bqk_sb")
        bv_sb = const.tile([P, HPC, DH], bf16, name="bv_sb")
        mask2_sb = const.tile([P, 2, P], bf16, name="mask2_sb")
        v_sb = const.tile([P, NT, HPC, DH + 1], bf16, name="v_sb")
        warm = const.tile([P, 2], f32, name="warm")
        qT = [const.tile([P, T], bf16, name=f"qT{p}") for p in range(2)]
        kT = [const.tile([P, T], bf16, name=f"kT{p}") for p in range(2)]
        yT = [const.tile([P, T], bf16, name=f"yT{p}") for p in range(2)]

        # --- loads: wqk/x(slab0)/wv per-chunk transfers interleaved so
        # compute starts early and no single-queue transfer serializes a
        # consumer; x slabs 1-3 last. bqkT/bvr/mask2 are host-prepped (a
        # transposing or broadcasting DMA descriptor is pathologically
        # slow: 128 tiny descriptors) ---
        wqkv = wqk.rearrange("(k p) n -> k p n", p=P)
        xTv = xT.rearrange("(k p) t -> k p t", p=P)
        wvv = wv.rearrange("(k p) n -> k p n", p=P)
        for k in range(CK):
            nc.sync.dma_start(out=wqk_sb[:, k, :], in_=wqkv[k])
            nc.sync.dma_start(out=x_ch[0][:, k, :], in_=xTv[k][:, 0:SLAB])
        for k in range(CK):
            nc.sync.dma_start(out=wv_sb[:, k, :], in_=wvv[k])
        nc.sync.dma_start(out=bqk_sb[:], in_=bqkT)
        nc.sync.dma_start(
            out=bv_sb[:], in_=bvr.rearrange("p (h d) -> p h d", d=DH)
        )
        nc.sync.dma_start(
            out=mask2_sb[:], in_=mask2.rearrange("p (h n) -> p h n", n=P)
        )
        nc.sync.dma_start(out=wo_sb[:], in_=wo.rearrange("(r p) n -> p r n", p=P))
        for c in range(1, NSL):
            for k in range(CK):
                nc.sync.dma_start(
                    out=x_ch[c][:, k, :],
                    in_=xTv[k][:, c * SLAB : (c + 1) * SLAB],
                )

        # scratch init + ACT exp-table preload during the initial DMA wait
        nc.vector.memset(warm[:, 0:1], 0.0)
        nc.scalar.activation(
            out=warm[:, 1:2], in_=warm[:, 0:1], func=AF.Exp,
            scale=float(1.0 / np.sqrt(DH)),
        )
        nc.vector.memset(v_sb[:, :, :, DH : DH + 1], 1.0)

        # --- filler quanta ----------------------------------------------
        def qk_block(s, blk):
            """QKV projection for one (pair, q/k) column block of slab s.
            Bias apply + PSUM->SBUF copy on DVE (ACT is exp-bound)."""
            p, qk = divmod(blk, 2)
            dst = qT[p] if qk == 0 else kT[p]
            ps = stp.tile([P, 2 * SLAB], f32, name="ps_qkv", tag="big")[:, :SLAB]
            for k in range(CK):
                nc.tensor.matmul(
                    ps[:],
                    lhsT=wqk_sb[:, k, blk * P : (blk + 1) * P],
                    rhs=x_ch[s][:, k, :],
                    start=(k == 0),
                    stop=(k == CK - 1),
                )
            nc.vector.tensor_scalar_add(
                out=dst[:, s * SLAB : (s + 1) * SLAB],
                in0=ps[:],
                scalar1=bqk_sb[:, blk : blk + 1],
            )

        def v_tile(s, tt):
            """V projection for one token tile (natural layout, ones
            column at DH preset by memset)."""
            t = 4 * s + tt
            ps = stp.tile([P, 2 * SLAB], f32, name="ps_v", tag="big")[
                :, : HPC * DH
            ]
            for k in range(CK):
                nc.tensor.matmul(
                    ps[:],
                    lhsT=x_ch[s][:, k, tt * P : (tt + 1) * P],
                    rhs=wv_sb[:, k, :],
                    start=(k == 0),
                    stop=(k == CK - 1),
                )
            nc.vector.tensor_add(
                out=v_sb[:, t, :, 0:DH],
                in0=ps[:].rearrange("p (h d) -> p h d", d=DH),
                in1=bv_sb[:],
            )

        def proj_tile(s, tt):
            """Output projection for one token tile of slab s (partial
            over this core's heads; host adds b_proj and reduces)."""
            t = 4 * s + tt
            pso = stp.tile([P, 2 * SLAB], f32, name="pso", tag="big")
            for ns in range(2):
                for p in range(2):
                    nc.tensor.matmul(
                        pso[:, ns * SLAB : (ns + 1) * SLAB],
                        lhsT=yT[p][:, t * P : (t + 1) * P],
                        rhs=wo_sb[:, p, ns * SLAB : (ns + 1) * SLAB],
                        start=(p == 0),
                        stop=(p == 1),
                    )
            ob = outp.tile([P, 2 * SLAB], bf16, name="ob", tag="ob")
            nc.vector.tensor_copy(out=ob[:], in_=pso[:])
            nc.sync.dma_start(out=out[t * P : (t + 1) * P, :], in_=ob[:])

        def attn(p, s, fillers, n_warm=2):
            """Causal attention for head pair p, tq slab s.

            tk order puts the diagonal tiles first: the first executed tk
            (4s) covers the full [0:SLAB] psum width with start=True, the
            other diagonal tiles accumulate partial widths, then the full
            tiles 0..4s-1. ST runs 3 tiles ahead and exp 2 ahead of PV so
            the diagonal mask multiplies never stall PV. The first
            `n_warm` fillers are emitted right after the warmup; the rest
            are spread evenly through the steady loop to keep the PE fed
            while ACT runs exp.
            """
            ntk = 4 * s + 4
            tk_order = [4 * s + i for i in range(4)] + list(range(4 * s))
            psy = [
                yp.tile([P, SLAB], f32, name=f"psy{hp}", tag="psy")
                for hp in range(2)
            ]

            def off_of(tk):
                diag_r = tk - 4 * s
                return diag_r * P if diag_r >= 0 else 0

            pend = {}
            exd = {}

            def st(i):
                tk = tk_order[i]
                off = off_of(tk)
                pp = stp.tile([P, 2 * SLAB], f32, name="pp", tag="big")
                for hp in range(2):
                    nc.tensor.matmul(
                        pp[:, hp * SLAB + off : (hp + 1) * SLAB],
                        lhsT=kT[p][hp * DH : (hp + 1) * DH, tk * P : (tk + 1) * P],
                        rhs=qT[p][hp * DH : (hp + 1) * DH, s * SLAB + off : (s + 1) * SLAB],
                        start=True,
                        stop=True,
                    )
                pend[i] = pp

            def do_exp(i):
                tk = tk_order[i]
                off = off_of(tk)
                pp = pend.pop(i)
                ex = expp.tile([P, 2 * SLAB], bf16, name="ex", tag="ex")
                ppv = pp[:].rearrange("q (h n) -> q h n", h=2)[:, :, off:]
                exv = ex[:].rearrange("q (h n) -> q h n", h=2)[:, :, off:]
                nc.scalar.activation(
                    out=exv,
                    in_=ppv,
                    func=AF.Exp,
                    scale=float(1.0 / np.sqrt(DH)),
                )
                if tk - 4 * s >= 0:
                    exd2 = ex[:].rearrange("q (h n) -> q h n", h=2)[
                        :, :, off : off + P
                    ]
                    nc.vector.tensor_mul(out=exd2, in0=exd2, in1=mask2_sb[:])
                exd[i] = ex

            for i in range(min(3, ntk)):
                st(i)
            do_exp(0)
            if ntk > 1:
                do_exp(1)
            n_warm = min(n_warm, len(fillers))
            for f in fillers[:n_warm]:
                f()
            rest = fillers[n_warm:]
            pos = {}
            for j in range(len(rest)):
                pos.setdefault((j + 1) * ntk // (len(rest) + 1), []).append(
                    rest[j]
                )
            for i in range(ntk):
                tk = tk_order[i]
                off = off_of(tk)
                if i + 3 < ntk:
                    st(i + 3)
                if i + 2 < ntk:
                    do_exp(i + 2)
                ex = exd.pop(i)
                for hp in range(2):
                    nc.tensor.matmul(
                        psy[hp][0 : DH + 1, off:],
                        lhsT=v_sb[:, tk, 2 * p + hp, :],
                        rhs=ex[:, hp * SLAB + off : (hp + 1) * SLAB],
                        start=(i == 0),
                        stop=(i == ntk - 1),
                    )
                for f in pos.get(i, []):
                    f()
            for hp in range(2):
                sm = rbp.tile([1, SLAB], f32, name="sm", tag="sm")
                nc.vector.tensor_copy(out=sm[:], in_=psy[hp][DH : DH + 1, :])
                rec = rbp.tile([1, SLAB], f32, name="rec", tag="rec")
                nc.vector.reciprocal_approx_fast(out=rec[:], in_=sm[:])
                rb = rbp.tile([DH, SLAB], f32, name="rb", tag="rb")
                nc.gpsimd.partition_broadcast(out_ap=rb[:], in_ap=rec[:])
                nc.vector.tensor_mul(
                    out=yT[p][hp * DH : (hp + 1) * DH, s * SLAB : (s + 1) * SLAB],
                    in0=psy[hp][0:DH, :],
                    in1=rb[:],
                )

        # --- fused schedule ---------------------------------------------
        # QKV blocks / V tiles / proj tiles of neighbouring slabs ride as
        # filler inside the ACT-bound attention loops.
        Q, V, O = qk_block, v_tile, proj_tile
        qk_block(0, 0)
        qk_block(0, 1)
        v_tile(0, 0)
        v_tile(0, 1)
        attn(0, 0, [lambda: V(0, 2), lambda: V(0, 3),
                    lambda: Q(0, 2), lambda: Q(0, 3)])
        attn(1, 0, [lambda: Q(1, 0), lambda: Q(1, 1)])
        v_tile(1, 0)
        v_tile(1, 1)
        v_tile(1, 2)
        v_tile(1, 3)
        attn(0, 1, [lambda: O(0, 0), lambda: O(0, 1), lambda: O(0, 2),
                    lambda: O(0, 3), lambda: Q(1, 2), lambda: Q(1, 3)])
        attn(1, 1, [lambda: Q(2, 0), lambda: Q(2, 1)])
        v_tile(2, 0)
        v_tile(2, 1)
        v_tile(2, 2)
        v_tile(2, 3)
        attn(0, 2, [lambda: O(1, 0), lambda: O(1, 1), lambda: O(1, 2),
                    lambda: O(1, 3), lambda: Q(2, 2), lambda: Q(2, 3)])
        attn(1, 2, [lambda: Q(3, 0), lambda: Q(3, 1)])
        v_tile(3, 0)
        v_tile(3, 1)
        v_tile(3, 2)
        v_tile(3, 3)
        attn(0, 3, [lambda: O(2, 0), lambda: O(2, 1),
                    lambda: Q(3, 2), lambda: Q(3, 3)])
        attn(1, 3, [lambda: O(2, 2), lambda: O(2, 3)])
        proj_tile(3, 0)
        proj_tile(3, 1)
        proj_tile(3, 2)
        proj_tile(3, 3)

    nc.compile()
    return nc


def get_program():
    if "nc" not in _CACHE:
        _CACHE["nc"] = _build_program()
    return _CACHE["nc"]


def make_core_inputs(x, w_attn, b_attn, w_proj, core):
    """Host-side shard preparation for one core (bf16 device I/O)."""
    import ml_dtypes

    bf16 = ml_dtypes.bfloat16
    b = core // 4
    g = core % 4
    heads = [4 * g + i for i in range(HPC)]

    xT = np.ascontiguousarray(np.asarray(x[b], np.float32).T).astype(bf16)

    def qcols(h):
        return w_attn[:, h * DH : (h + 1) * DH]

    def kcols(h):
        return w_attn[:, C + h * DH : C + (h + 1) * DH]

    def vcols(h):
        return w_attn[:, 2 * C + h * DH : 2 * C + (h + 1) * DH]

    h0, h1, h2, h3 = heads
    wqk = np.ascontiguousarray(
        np.concatenate(
            [qcols(h0), qcols(h1), kcols(h0), kcols(h1),
             qcols(h2), qcols(h3), kcols(h2), kcols(h3)],
            axis=1,
        )
    ).astype(bf16)
    wv = np.ascontiguousarray(
        np.concatenate([vcols(h) for h in heads], axis=1)
    ).astype(bf16)
    bqkT = np.stack(
        [
            np.concatenate([b_attn[h0 * DH : (h0 + 1) * DH], b_attn[h1 * DH : (h1 + 1) * DH]]),
            np.concatenate([b_attn[C + h0 * DH : C + (h0 + 1) * DH], b_attn[C + h1 * DH : C + (h1 + 1) * DH]]),
            np.concatenate([b_attn[h2 * DH : (h2 + 1) * DH], b_attn[h3 * DH : (h3 + 1) * DH]]),
            np.concatenate([b_attn[C + h2 * DH : C + (h2 + 1) * DH], b_attn[C + h3 * DH : C + (h3 + 1) * DH]]),
        ],
        axis=1,
    ).astype(np.float32)
    bv = np.concatenate(
        [b_attn[2 * C + h * DH : 2 * C + (h + 1) * DH] for h in heads]
    ).astype(bf16)
    bvr = np.broadcast_to(bv[None, :], (P, HPC * DH))
    wo = np.ascontiguousarray(
        w_proj[heads[0] * DH : (heads[-1] + 1) * DH, :]
    ).astype(bf16)

    mask = np.triu(np.ones((P, P))).astype(bf16)
    mask2 = np.concatenate([mask, mask], axis=1)
    return {
        "xT": xT,
        "wqk": wqk,
        "wv": wv,
        "wo": wo,
        "bqkT": np.ascontiguousarray(bqkT),
        "bvr": np.ascontiguousarray(bvr),
        "mask2": np.ascontiguousarray(mask2),
    }


def kernel(x, w_attn, b_attn, w_proj, b_proj):
    from concourse.bass_utils import run_bass_kernel_spmd

    x = np.asarray(x, np.float32)
    w_attn = np.asarray(w_attn, np.float32)
    b_attn = np.asarray(b_attn, np.float32)
    w_proj = np.asarray(w_proj, np.float32)
    b_proj = np.asarray(b_proj, np.float32)

    nc = get_program()
    in_maps = [
        make_core_inputs(x, w_attn, b_attn, w_proj, core) for core in range(NCORES)
    ]
    res = run_bass_kernel_spmd(nc, in_maps, core_ids=list(range(NCORES)))
    outs = [np.asarray(m["out"], np.float32) for m in res.results]

    y = np.empty((B, T, C), np.float32)
    for b in range(B):
        y[b] = outs[4 * b] + outs[4 * b + 1] + outs[4 * b + 2] + outs[4 * b + 3]
        y[b] += b_proj[None, :]
    return y
